# revision 26
# baseline (speedup 1.0000x reference)
"""Cosine multihead attention on 8 Trainium2 NeuronCores.

Sharding: batch*heads across cores. Core c handles batch b = c // 4 and the
4 heads [4*(c%4), 4*(c%4)+4). Each core computes its heads' q/k/v projections
(tensor-parallel slices of in_proj), full attention for its (B,H) slice, and a
partial out-projection (rank-256 contribution, bf16). The host sums the 4
partials per batch in fp32 and adds out_proj_bias.

Structure (v4):
- q/k/v projected transposed [dims, seq] with 8x128 contraction chunks;
  biases are folded into the PSUM drain (tensor_scalar add with a
  per-partition bias vector) instead of extra matmuls.
- All projection matmuls are emitted first (PE stays dense); the norm chains
  (square/row-sum/sqrt/reciprocal) run afterwards on ACT/DVE, pair 0 first.
- k is NOT normalized per-vector: kt is pre-scaled in place by 1/(tau*||k||)
  (PE selector-broadcast of the reciprocal row + one DVE multiply), so the
  softmax exp needs no scale operand.
- v lands transposed, then one DMA-transpose per 128-dim block into natural
  [keys, kc, head, 65] layout; column 64 holds ones so PV also accumulates
  softmax denominators.
- Attention in 512-query rounds: sc [128 keys, 2x512 q] double-buffered;
  QK(kc) pair on concurrent 64-row PE tiles; PV(kc-1) emitted after QK(kc) so
  the in-order PE never waits on the current exp. exp on ACT, with a subset
  of kc chunks computed on the DVE via a Schraudolph bf16 bit-trick
  (t = s*C + B; round to int16; the bits are the bf16 exp).
- Out-projection units (512-wide halves, 1-bank PSUM) interleave into the
  pair-1 attention rounds; output partial is bf16.
"""

import sys

if "/opt/trn_rl_repo" not in sys.path:
    sys.path.insert(0, "/opt/trn_rl_repo")

import numpy as np
import ml_dtypes

import concourse.bass as bass
import concourse.tile as tile
from concourse import bacc, mybir
from concourse.bass_utils import run_bass_kernel_spmd

S, B, E, H = 2048, 2, 1024, 16
HD = E // H            # 64
HPC = 4                # heads per core
NCORES = 8
TAU_MIN = 0.01

BF16 = ml_dtypes.bfloat16
DT_BF = mybir.dt.bfloat16
DT_F32 = mybir.dt.float32
DT_I16 = mybir.dt.int16

KC_E = E // 128        # 8 contraction chunks for projections
NPAIR = HPC // 2       # 2 head pairs per core
NKC = S // 128         # 16 key chunks in attention

# Schraudolph exp-on-DVE: which kc chunks of each round go to the vector
# engine instead of ACT.
SCHR_KC = frozenset()
SCHR_C = 128.0 * 1.4426950408889634   # 2^7 * log2(e)
SCHR_B = 127.0 * 128.0 - 6.0          # exponent bias - error-balancing shift


def build_program():
    nc = bacc.Bacc(None)

    xq = nc.dram_tensor("xq_t", [E, S], DT_BF, kind="ExternalInput")
    xk = nc.dram_tensor("xk_t", [E, S], DT_BF, kind="ExternalInput")
    xv = nc.dram_tensor("xv_t", [E, S], DT_BF, kind="ExternalInput")
    wq = nc.dram_tensor("wq_t", [E, 256], DT_BF, kind="ExternalInput")
    wk = nc.dram_tensor("wk_t", [E, 256], DT_BF, kind="ExternalInput")
    wv = nc.dram_tensor("wv_t", [E, 256], DT_BF, kind="ExternalInput")
    bq = nc.dram_tensor("b_q", [128, 2], DT_F32, kind="ExternalInput")
    bk = nc.dram_tensor("b_k", [128, 2], DT_F32, kind="ExternalInput")
    bv = nc.dram_tensor("b_v", [128, 2], DT_F32, kind="ExternalInput")
    wo = nc.dram_tensor("wo_t", [256, E], DT_BF, kind="ExternalInput")
    selq_in = nc.dram_tensor("selq", [2, 128], DT_F32, kind="ExternalInput")
    tau2_in = nc.dram_tensor("tau2i", [2, 2], DT_F32, kind="ExternalInput")
    outp = nc.dram_tensor("out_p", [S, E], DT_BF, kind="ExternalOutput")
    DEBUG = bool(__import__("os").environ.get("KDEBUG"))
    if DEBUG:
        dbg_qt = nc.dram_tensor("dbg_qt", [2, 128, S], DT_BF, kind="ExternalOutput")
        dbg_kt = nc.dram_tensor("dbg_kt", [2, 128, S], DT_BF, kind="ExternalOutput")
        dbg_rk2 = nc.dram_tensor("dbg_rk2", [2, NPAIR, S], DT_BF, kind="ExternalOutput")
        dbg_v = nc.dram_tensor("dbg_v", [128, 16, 4, 65], DT_BF, kind="ExternalOutput")
        dbg_ht = nc.dram_tensor("dbg_ht", [2, 128, S], DT_BF, kind="ExternalOutput")

    with tile.TileContext(nc) as tc:
        with (
            tc.tile_pool(name="consts", bufs=1) as consts,
            tc.tile_pool(name="xin", bufs=1) as xin,
            tc.tile_pool(name="xvp", bufs=1) as xvp,
            tc.tile_pool(name="wts", bufs=1) as wts,
            tc.tile_pool(name="qk", bufs=1) as qkpool,
            tc.tile_pool(name="norm", bufs=1) as normpool,
            tc.tile_pool(name="work", bufs=2) as work,
            tc.tile_pool(name="sqp", bufs=2) as sqp,
            tc.tile_pool(name="expool", bufs=4) as expool,
            tc.tile_pool(name="zwork", bufs=1) as zwork,
            tc.tile_pool(name="outs", bufs=2) as outs,
            tc.tile_pool(name="ps_a", bufs=2, space="PSUM") as ps_a,
            tc.tile_pool(name="ps_o", bufs=1, space="PSUM") as ps_o,
            tc.tile_pool(name="ps_c", bufs=2, space="PSUM") as ps_c,
        ):
            # ---- constants -------------------------------------------------
            selq = consts.tile([2, 128], DT_F32, tag="selq")
            nc.sync.dma_start(out=selq, in_=selq_in[:, :])
            tau2_sb = consts.tile([2, 2], DT_F32, tag="tau2")
            nc.sync.dma_start(out=tau2_sb, in_=tau2_in[:, :])
            bq_sb = consts.tile([128, 2], DT_F32, tag="bq")
            bk_sb = consts.tile([128, 2], DT_F32, tag="bk")
            bv_sb = consts.tile([128, 2], DT_F32, tag="bv")
            nc.sync.dma_start(out=bq_sb, in_=bq[:, :])
            nc.sync.dma_start(out=bk_sb, in_=bk[:, :])
            nc.sync.dma_start(out=bv_sb, in_=bv[:, :])
            hsel = consts.tile([128, 2], DT_BF, tag="hsel")
            nc.vector.memset(hsel, 0.0)
            nc.vector.memset(hsel[0:64, 0:1], 1.0)
            nc.vector.memset(hsel[64:128, 1:2], 1.0)
            ones_hi = consts.tile([128, 64], DT_F32, tag="ones_hi")
            nc.vector.memset(ones_hi, 1.0)
            selqb = consts.tile([2, 128], DT_BF, tag="selqb")
            nc.vector.tensor_copy(out=selqb, in_=selq)

            # ---- weights ---------------------------------------------------
            wq_sb = wts.tile([128, KC_E, 256], DT_BF, tag="wq")
            wk_sb = wts.tile([128, KC_E, 256], DT_BF, tag="wk")
            wv_sb = wts.tile([128, KC_E, 256], DT_BF, tag="wv")
            wo_sb = wts.tile([128, 2, E], DT_BF, tag="wo")

            # ---- activations: per-chunk tiles so projections can start on
            # the first chunk; xq split across both HWDGE queues and placed
            # at the head of each queue (weights interleave after xq0/xq1).
            xq_c = [xin.tile([128, S], DT_BF, tag=f"xq{c}", name=f"xq{c}")
                    for c in range(KC_E)]
            xk_c = [xin.tile([128, S], DT_BF, tag=f"xk{c}", name=f"xk{c}")
                    for c in range(KC_E)]
            nc.sync.dma_start(out=xq_c[0], in_=xq[0:128, :])
            nc.scalar.dma_start(out=xq_c[1], in_=xq[128:256, :])
            for c in range(KC_E):
                nc.gpsimd.dma_start(out=wk_sb[:, c, :], in_=wk[c * 128:(c + 1) * 128, :])
            for c in range(KC_E):
                eng = nc.sync if c % 2 == 0 else nc.scalar
                eng.dma_start(out=wq_sb[:, c, :], in_=wq[c * 128:(c + 1) * 128, :])
            for c in range(2, KC_E):
                eng = nc.sync if c % 2 == 0 else nc.scalar
                eng.dma_start(out=xq_c[c], in_=xq[c * 128:(c + 1) * 128, :])
            for c in range(KC_E):
                eng = nc.scalar if c % 2 == 0 else nc.sync
                eng.dma_start(out=xk_c[c], in_=xk[c * 128:(c + 1) * 128, :])
                nc.gpsimd.dma_start(out=wv_sb[:, c, :], in_=wv[c * 128:(c + 1) * 128, :])
            # xv shares its pool slot with v_sb (xv is dead once the
            # transposed v projection has consumed it).
            xv_sb = xvp.tile([128, KC_E, S], DT_BF, tag="xv", name="xv_sb")
            for c in range(KC_E):
                eng = nc.scalar if c % 2 == 0 else nc.sync
                eng.dma_start(out=xv_sb[:, c, :], in_=xv[c * 128:(c + 1) * 128, :])
            for c in range(2):
                nc.gpsimd.dma_start(out=wo_sb[:, c, :], in_=wo[c * 128:(c + 1) * 128, :])

            qt = [qkpool.tile([128, S], DT_BF, tag=f"qt{p}", name=f"qt{p}")
                  for p in range(NPAIR)]
            kt = [qkpool.tile([128, S], DT_BF, tag=f"kt{p}", name=f"kt{p}")
                  for p in range(NPAIR)]
            heads_t = [qkpool.tile([128, S], DT_BF, tag=f"ht{p}", name=f"ht{p}")
                       for p in range(NPAIR)]
            vT_sb = qkpool.tile([128, 2, S], DT_BF, tag="vT")

            # per-key reciprocal norms 1/(tau*||k||), as rows (bf16)
            rk2 = normpool.tile([2, NPAIR, S], DT_BF, tag="rk2")

            def proj_unit(dst, w_sb, b_sb, x_sb, mcol, half):
                """16 accumulating matmuls + bias-adding PSUM drain.
                x_sb is either a list of per-chunk [128,S] tiles or one
                [128,KC_E,S] tile."""
                sl = slice(half * 1024, (half + 1) * 1024)
                pp = ps_a.tile([128, 1024], DT_F32, tag="a", name="pp")
                for c in range(KC_E):
                    xc = x_sb[c] if isinstance(x_sb, list) else x_sb[:, c, :]
                    for hh in range(2):
                        nc.tensor.matmul(
                            pp[:, hh * 512:(hh + 1) * 512],
                            lhsT=w_sb[:, c, mcol * 128:(mcol + 1) * 128],
                            rhs=xc[:,
                                   half * 1024 + hh * 512:
                                   half * 1024 + (hh + 1) * 512],
                            start=(c == 0),
                            stop=(c == KC_E - 1),
                        )
                nc.vector.tensor_scalar(
                    out=dst[:, sl], in0=pp,
                    scalar1=b_sb[:, mcol:mcol + 1], scalar2=None,
                    op0=mybir.AluOpType.add,
                )

            def rowsum_sq(src_sl, name):
                """DVE square + PE row-sum -> two 1-bank [2,512] psum tiles."""
                sq = sqp.tile([128, 1024], DT_BF, tag="sq", name=f"sq_{name}")
                nc.vector.tensor_mul(sq, src_sl, src_sl)
                ssh = []
                for hh in range(2):
                    ss = ps_c.tile([2, 512], DT_F32, tag="c", name=f"ss_{name}")
                    nc.tensor.matmul(
                        ss, lhsT=hsel, rhs=sq[:, hh * 512:(hh + 1) * 512],
                        start=True, stop=True)
                    ssh.append(ss)
                return ssh

            def q_norm(mc, half):
                sl = slice(half * 1024, (half + 1) * 1024)
                ssh = rowsum_sq(qt[mc][:, sl], "q")
                st = work.tile([2, 1024], DT_F32, tag="st", name="st_q")
                for hh in range(2):
                    nc.scalar.activation(
                        st[:, hh * 512:(hh + 1) * 512], ssh[hh],
                        mybir.ActivationFunctionType.Sqrt)
                for hh in range(2):
                    rb = ps_c.tile([128, 512], DT_F32, tag="c", name="rb_q")
                    nc.tensor.matmul(
                        rb, lhsT=selq, rhs=st[:, hh * 512:(hh + 1) * 512],
                        start=True, stop=True)
                    rq = work.tile([128, 512], DT_F32, tag="rq", name="rq_q")
                    nc.vector.reciprocal_approx_fast(out=rq, in_=rb)
                    s2 = slice(half * 1024 + hh * 512,
                               half * 1024 + (hh + 1) * 512)
                    nc.vector.tensor_mul(qt[mc][:, s2], qt[mc][:, s2], rq)

            def k_norm(mc, half):
                sl = slice(half * 1024, (half + 1) * 1024)
                ssh = rowsum_sq(kt[mc][:, sl], "k")
                st = work.tile([2, 1024], DT_F32, tag="st", name="st_k")
                for hh in range(2):
                    # sqrt(ss * tau^2) = tau * ||k||
                    nc.scalar.activation(
                        st[:, hh * 512:(hh + 1) * 512], ssh[hh],
                        mybir.ActivationFunctionType.Sqrt,
                        scale=tau2_sb[:, mc:mc + 1])
                rr = work.tile([2, 1024], DT_F32, tag="st", name="rr_k")
                nc.vector.reciprocal_approx_fast(out=rr, in_=st)
                nc.vector.tensor_copy(out=rk2[:, mc, sl], in_=rr)

            def k_prescale(p, half):
                # PE-broadcast the reciprocal-norm rows over head partitions,
                # then scale kt in place (DVE reads the PSUM broadcast).
                for hh in range(2):
                    s2 = slice(half * 1024 + hh * 512,
                               half * 1024 + (hh + 1) * 512)
                    rbk = ps_c.tile([128, 512], DT_F32, tag="c", name="rbk")
                    nc.tensor.matmul(
                        rbk, lhsT=selqb, rhs=rk2[:, p, s2],
                        start=True, stop=True)
                    nc.vector.tensor_mul(kt[p][:, s2], kt[p][:, s2], rbk)

            # ---- phase 1: all projection matmuls (PE dense) ----------------
            for half in range(2):
                proj_unit(qt[0], wq_sb, bq_sb, xq_c, 0, half)
            for half in range(2):
                proj_unit(kt[0], wk_sb, bk_sb, xk_c, 0, half)
            for half in range(2):
                proj_unit(qt[1], wq_sb, bq_sb, xq_c, 1, half)
            for half in range(2):
                proj_unit(kt[1], wk_sb, bk_sb, xk_c, 1, half)
            for d in range(2):
                for half in range(2):
                    proj_unit(vT_sb[:, d, :], wv_sb, bv_sb, xv_sb, d, half)

            # ---- phase 2: norm chains, pair 0 first ------------------------
            v_sb = xvp.tile([128, NKC, HPC, HD + 1], DT_BF, tag="xv", name="v_sb")
            nc.gpsimd.memset(v_sb[:, :, :, HD:HD + 1], 1.0)

            def v_transpose(d):
                v2 = work.tile([128, NKC, 128], DT_BF, tag="rq", name="v2")
                nc.sync.dma_start_transpose(out=v2, in_=vT_sb[:, d, :])
                for j in range(2):
                    nc.vector.tensor_copy(
                        out=v_sb[:, :, 2 * d + j, 0:HD],
                        in_=v2[:, :, j * 64:(j + 1) * 64],
                    )

            for half in range(2):
                q_norm(0, half)
            for half in range(2):
                k_norm(0, half)
            for half in range(2):
                k_prescale(0, half)
            v_transpose(0)
            for half in range(2):
                q_norm(1, half)
            for half in range(2):
                k_norm(1, half)
            for half in range(2):
                k_prescale(1, half)
            v_transpose(1)

            # ---- attention -------------------------------------------------
            def attention_round(p, qb, fillers=()):
                fillers = list(fillers)
                sl_q = slice(qb * 512, (qb + 1) * 512)
                o = ps_o.tile([65, 1024], DT_F32, tag="o", name="o_acc")
                exs = [None] * NKC

                def qk(kc):
                    scn = ps_a.tile([128, 1024], DT_F32, tag="a", name="sc")
                    for j in range(2):
                        rows = slice(j * 64, (j + 1) * 64)
                        nc.tensor.matmul(
                            scn[:, j * 512:(j + 1) * 512],
                            lhsT=kt[p][rows, kc * 128:(kc + 1) * 128],
                            rhs=qt[p][rows, sl_q],
                            start=True, stop=True,
                        )
                    ex = expool.tile([128, 1024], DT_BF, tag="ex", name="ex")
                    if kc in SCHR_KC:
                        tf = work.tile([128, 1024], DT_F32, tag="rq", name="tf")
                        nc.vector.tensor_scalar(
                            out=tf, in0=scn,
                            scalar1=SCHR_C, scalar2=SCHR_B,
                            op0=mybir.AluOpType.mult,
                            op1=mybir.AluOpType.add,
                        )
                        nc.vector.tensor_copy(out=ex.bitcast(DT_I16), in_=tf)
                    else:
                        nc.scalar.activation(
                            ex, scn, mybir.ActivationFunctionType.Exp
                        )
                    exs[kc] = ex

                def pv(kc):
                    for j in range(2):
                        nc.tensor.matmul(
                            o[0:65, j * 512:(j + 1) * 512],
                            lhsT=v_sb[:, kc, 2 * p + j, :],
                            rhs=exs[kc][:, j * 512:(j + 1) * 512],
                            start=(kc == 0), stop=(kc == NKC - 1),
                        )

                stride = max(1, NKC // (len(fillers) + 1)) if fillers else NKC
                for kc in range(NKC):
                    qk(kc)
                    if kc > 0:
                        pv(kc - 1)
                    if fillers and kc % stride == stride - 1:
                        fillers.pop(0)()
                pv(NKC - 1)

                # normalize: z row (partition 64) -> SBUF (scalar engine),
                # PE-broadcast to partitions 0-63, reciprocal, multiply.
                zs = zwork.tile([128, 1024], DT_F32, tag="rz", name="zs")
                nc.vector.tensor_copy(zs[64:65, :], o[64:65, :])
                for hh in range(2):
                    zb = ps_c.tile([64, 512], DT_F32, tag="c", name="zb")
                    nc.tensor.matmul(
                        zb,
                        lhsT=ones_hi[64:65, 0:64],
                        rhs=zs[64:65, hh * 512:(hh + 1) * 512],
                        start=True, stop=True,
                    )
                    zbi = zwork.tile([64, 512], DT_F32, tag="zb", name="zbi")
                    nc.vector.reciprocal_approx_fast(out=zbi, in_=zb)
                    j = hh  # head j's queries live in free half hh
                    nc.vector.tensor_mul(
                        heads_t[p][j * 64:(j + 1) * 64, sl_q],
                        o[0:64, hh * 512:(hh + 1) * 512],
                        zbi,
                    )

            def outproj_half(m, eh):
                def go():
                    op = ps_c.tile([128, 512], DT_F32, tag="c", name="op")
                    for c in range(2):
                        nc.tensor.matmul(
                            op,
                            lhsT=heads_t[c][:, m * 128:(m + 1) * 128],
                            rhs=wo_sb[:, c, eh * 512:(eh + 1) * 512],
                            start=(c == 0), stop=(c == 1),
                        )
                    ob = outs.tile([128, 512], DT_BF, tag="ob", name="ob")
                    nc.vector.tensor_copy(ob, op)
                    nc.sync.dma_start(
                        out=outp[m * 128:(m + 1) * 128,
                                 eh * 512:(eh + 1) * 512],
                        in_=ob)
                return go

            if DEBUG:
                for p in range(NPAIR):
                    nc.sync.dma_start(out=dbg_qt[p], in_=qt[p][:, :])
                    nc.sync.dma_start(out=dbg_kt[p], in_=kt[p][:, :])
                nc.sync.dma_start(out=dbg_rk2[:, :, :], in_=rk2[:, :, :])
                nc.sync.dma_start(out=dbg_v[:, :, :, :], in_=v_sb[:, :, :, :])

            for qb in range(4):
                attention_round(0, qb)
            for qb in range(4):
                # interleave the previous round's out-projection chunks
                if qb == 0:
                    fill = []
                else:
                    ms = range(4 * (qb - 1), 4 * qb)
                    fill = [outproj_half(m, eh) for m in ms for eh in range(2)][:4]
                    fill2 = [outproj_half(m, eh) for m in ms for eh in range(2)][4:]
                attention_round(1, qb, fillers=fill if qb else ())
                if qb:
                    for f in fill2:
                        f()
            for m in range(12, 16):
                for eh in range(2):
                    outproj_half(m, eh)()
            if DEBUG:
                for p in range(NPAIR):
                    nc.sync.dma_start(out=dbg_ht[p], in_=heads_t[p][:, :])
    nc.compile()
    return nc


_CACHE = {}


def _get_program():
    if "nc" not in _CACHE:
        _CACHE["nc"] = build_program()
    return _CACHE["nc"]


def make_in_maps(query, key, value, in_proj_weight, in_proj_bias,
                 out_proj_weight, out_proj_bias, tau):
    query = np.asarray(query, np.float32)
    key = np.asarray(key, np.float32)
    value = np.asarray(value, np.float32)
    W = np.asarray(in_proj_weight, np.float32)
    bias = np.asarray(in_proj_bias, np.float32)
    Wo = np.asarray(out_proj_weight, np.float32)
    tau_c = np.maximum(np.asarray(tau, np.float32).reshape(H), TAU_MIN)

    xT = {}
    for b in range(B):
        xT["q", b] = np.ascontiguousarray(query[:, b, :].T).astype(BF16)
        xT["k", b] = np.ascontiguousarray(key[:, b, :].T).astype(BF16)
        xT["v", b] = np.ascontiguousarray(value[:, b, :].T).astype(BF16)

    selq_host = np.zeros((2, 128), np.float32)
    selq_host[0, 0:64] = 1.0
    selq_host[1, 64:128] = 1.0
    in_maps = []
    for c in range(NCORES):
        b = c // 4
        h0 = HPC * (c % 4)
        rows = slice(h0 * HD, (h0 + HPC) * HD)
        rows_k = slice(E + h0 * HD, E + (h0 + HPC) * HD)
        rows_v = slice(2 * E + h0 * HD, 2 * E + (h0 + HPC) * HD)
        # tau^2 per (head-in-pair, pair): sqrt(ss * tau^2) = tau * ||k||,
        # whose reciprocal is the exp scale 1/(tau*||k||).
        tau2i = np.zeros((2, 2), np.float32)
        for mc in range(NPAIR):
            tau2i[0, mc] = tau_c[h0 + 2 * mc] ** 2
            tau2i[1, mc] = tau_c[h0 + 2 * mc + 1] ** 2
        in_maps.append({
            "xq_t": xT["q", b],
            "xk_t": xT["k", b],
            "xv_t": xT["v", b],
            "wq_t": np.ascontiguousarray(W[rows, :].T).astype(BF16),
            "wk_t": np.ascontiguousarray(W[rows_k, :].T).astype(BF16),
            "wv_t": np.ascontiguousarray(W[rows_v, :].T).astype(BF16),
            # bias as [128 dims, 2 mc-chunks] per-partition columns
            "b_q": np.ascontiguousarray(bias[rows].reshape(2, 128).T),
            "b_k": np.ascontiguousarray(bias[rows_k].reshape(2, 128).T),
            "b_v": np.ascontiguousarray(bias[rows_v].reshape(2, 128).T),
            "wo_t": np.ascontiguousarray(Wo[:, rows].T).astype(BF16),
            "selq": selq_host,
            "tau2i": tau2i,
        })
    return in_maps


def assemble_out(results, out_proj_bias):
    bo = np.asarray(out_proj_bias, np.float32)
    out = np.zeros((S, B, E), np.float32)
    for c in range(NCORES):
        out[:, c // 4, :] += results[c]["out_p"].astype(np.float32)
    out += bo[None, None, :]
    return out


def kernel(query, key, value, in_proj_weight, in_proj_bias,
           out_proj_weight, out_proj_bias, tau):
    nc = _get_program()
    in_maps = make_in_maps(query, key, value, in_proj_weight, in_proj_bias,
                           out_proj_weight, out_proj_bias, tau)
    res = run_bass_kernel_spmd(nc, in_maps, core_ids=list(range(NCORES)))
    return assemble_out(res.results, out_proj_bias)


if __name__ == "__main__":
    import reference

    inputs = {k: np.asarray(v) for k, v in reference.setup_inputs().items()}
    out = kernel(**inputs)
    print("out shape", out.shape, out.dtype)
